# revision 1
# baseline (speedup 1.0000x reference)
"""Encoder-decoder GQA attention block (B=4, L=S=1024, H=2048, 32 Q heads,
8 KV heads, head_dim 64) + output projection + residual + layernorm, on 8
Trainium2 NeuronCores.

Sharding: rows. Core c handles batch c//2, L-half c%2 (512 query rows).
K/V projections are computed per-batch (duplicated on the 2 cores sharing a
batch — cheap), attention over all 32 heads for the core's rows, output
projection, residual + LN. No collectives.

Layout strategy: activations live feature-major ("transposed", H on the
partition dim) so every matmul uses naturally-laid-out weights as lhsT and
never needs an on-chip transpose:
  qT = Wq.T @ xT   (feature-major, per-partition bias add)
  kT = Wk.T @ eT   (feature-major)
  v  = (eT.T chunks as lhsT) @ Wv  (natural [s, 512]; free-dim bias add)
  scoresT[s, t] per head = kT_h.T @ qT_h  -> softmax over the partition (s)
  dim: exp via ACT (scale 1/8 fused), rowsum via an all-ones 65th column
  appended to V in the attn@V matmul; the reciprocal is broadcast across 64
  partitions with a K=1 matmul into the free upper half of the same PSUM
  bank. ctxT (feature-major) = V|1.T @ expT, then out = ctxT.T @ Wo lands
  back in natural [tok, H] layout for residual + layernorm (free-dim
  reductions).

Scheduling: Q-projection blocks are interleaved with attention per kv-head
pair so the ACT engine (softmax exp, ~133us) streams behind the PE the whole
time instead of saturating a separate attention phase; the last attn@V pair
of each kv-head pair is software-pipelined into the next block so its exp
tiles get a full production window. K/V projections are ordered to match
DMA arrival (K-low, V-low, K-high, V-high) on a dedicated 4-buffer PSUM
pool; input DMAs are k-granular so dependent chains start as chunks land;
the first Wo block and layernorm constants prefetch during attention; the
gamma/beta elementwise work runs on the otherwise-idle GpSimd engine in
phase C (except the final row block, where DVE is free). DMA issue order is
strict needed-first: the cost model (and aggregate HBM bandwidth on HW)
serializes transfers, so gamma/beta/Wo prefetches are deferred behind the
projection inputs. Cost model: ~322us/core; PE-busy floor ~280us (attention
matmuls are output-rate-bound at K=64/M=65, so the 280us floor is tight).

All matmuls bf16 (inputs pre-cast on host), fp32 PSUM accumulation, softmax
in fp32 (no max-subtraction: |scores| <= ~7 by construction, exp is safe in
fp32).
"""

from contextlib import ExitStack

import numpy as np
import ml_dtypes

import concourse.bass as bass  # noqa: F401  (bass.AP used via handles)
import concourse.mybir as mybir
import concourse.tile as tile
from concourse import bacc
from concourse.bass_utils import run_bass_kernel_spmd

BF16 = ml_dtypes.bfloat16

H = 2048
NH = 32
KVH = 8
G = 4           # query-head groups per kv head
HD = 64
B, L, S = 4, 1024, 1024
TOK = 512       # decoder rows per core
KC = H // 128   # 16 contraction chunks
SC = S // 128   # 8 s chunks
EPS = 1e-6

FP = mybir.dt.float32
BF = mybir.dt.bfloat16

_CACHE: dict = {}


def _build(use_mask: bool):
    nc = bacc.Bacc("TRN2", target_bir_lowering=False)

    xT = nc.dram_tensor("xT", [H, TOK], BF, kind="ExternalInput")
    xres = nc.dram_tensor("xres", [TOK, H], FP, kind="ExternalInput")
    eT = nc.dram_tensor("eT", [H, S], BF, kind="ExternalInput")
    wq = nc.dram_tensor("wq", [H, H], BF, kind="ExternalInput")
    wk = nc.dram_tensor("wk", [H, KVH * HD], BF, kind="ExternalInput")
    wv = nc.dram_tensor("wv", [H, KVH * HD], BF, kind="ExternalInput")
    wo = nc.dram_tensor("wo", [H, H], BF, kind="ExternalInput")
    bq2 = nc.dram_tensor("bq2", [128, KC], FP, kind="ExternalInput")
    bk2 = nc.dram_tensor("bk2", [128, 4], FP, kind="ExternalInput")
    bvr = nc.dram_tensor("bvr", [128, KVH * HD], FP, kind="ExternalInput")
    gamr = nc.dram_tensor("gamr", [128, H], BF, kind="ExternalInput")
    betr = nc.dram_tensor("betr", [128, H], BF, kind="ExternalInput")
    if use_mask:
        maskT = nc.dram_tensor("maskT", [S, TOK], BF, kind="ExternalInput")
    out = nc.dram_tensor("out", [TOK, H], FP, kind="ExternalOutput")

    Exp = mybir.ActivationFunctionType.Exp
    Sqrt = mybir.ActivationFunctionType.Sqrt

    eT4 = eT.rearrange("(j a p) s -> j p a s", a=4, p=128)
    xT4 = xT.rearrange("(j a p) s -> j p a s", a=4, p=128)
    wk4 = wk.rearrange("(j a p) n -> j p a n", a=4, p=128)
    wv4 = wv.rearrange("(j a p) n -> j p a n", a=4, p=128)
    wq4 = wq.rearrange("(j a p) n -> j p a n", a=4, p=128)
    wo4 = wo.rearrange("(j a p) n -> j p a n", a=4, p=128)

    with tile.TileContext(nc) as tc:
      with (
          tc.tile_pool(name="ctxT", bufs=KC) as ctxp,
          tc.tile_pool(name="cc", bufs=1) as ccp,
          tc.tile_pool(name="ln", bufs=10) as lnp,
      ):
        gam_sb = ccp.tile([128, H], BF, name="gam_sb")
        bet_sb = ccp.tile([128, H], BF, name="bet_sb")
        eps_sb = ccp.tile([128, 1], FP, name="eps_sb")
        nc.vector.memset(eps_sb[:], EPS)
        ctx_sb = [ctxp.tile([128, TOK], BF, tag="ctx", name="ctx")
                  for _ in range(KC)]

        wqp_cm = tc.tile_pool(name="wq", bufs=8)
        wqp = wqp_cm.__enter__()
        _stk = ExitStack()
        psA = _stk.enter_context(tc.tile_pool(name="psA", bufs=2, space="PSUM"))
        constp = _stk.enter_context(tc.tile_pool(name="const", bufs=1))
        xtp = _stk.enter_context(tc.tile_pool(name="xTp", bufs=4))
        qtp = _stk.enter_context(tc.tile_pool(name="qT", bufs=NH))
        ktp = _stk.enter_context(tc.tile_pool(name="kT", bufs=KVH))
        vvp = _stk.enter_context(tc.tile_pool(name="vv", bufs=SC))
        mkp = _stk.enter_context(
            tc.tile_pool(name="maskp", bufs=SC if use_mask else 1))

        qT_sb = [None] * NH
        kT_sb = [ktp.tile([64, S], BF, tag="kt", name="kt") for _ in range(KVH)]
        vv_sb = []

        with (
            tc.tile_pool(name="eTp", bufs=4) as etp,
            tc.tile_pool(name="wk", bufs=4) as wkp,
            tc.tile_pool(name="wv", bufs=4) as wvp,
            tc.tile_pool(name="psKV", bufs=4, space="PSUM") as psKV,
        ):
            # input DMAs, needed-first; 2-chunk granularity balances the
            # 565ns/DMA SP-SEQ issue cost against arrival granularity, and
            # the (tiny but 565ns each) bias DMAs issue after the first
            # critical transfers
            bq_sb = constp.tile([128, KC], FP, name="bq_sb")
            bk_sb = constp.tile([128, 4], FP, name="bk_sb")
            bv_sb = constp.tile([128, KVH * HD], FP, name="bv_sb")
            wk_sb, wv_sb, xT_sb = [], [], []
            et_tiles = []
            for j in range(4):
                t = wkp.tile([128, 4, 512], BF, tag="wkt", name="wkt")
                e = etp.tile([128, 4, S], BF, tag="et", name="et")
                for a in (0, 2):
                    nc.sync.dma_start(t[:, a:a + 2, :], wk4[j][:, a:a + 2, :])
                    nc.sync.dma_start(
                        e[:, a:a + 2, 0:512], eT4[j][:, a:a + 2, 0:512]
                    )
                wk_sb.extend(t[:, a, :] for a in range(4))
                et_tiles.append(e)
            nc.sync.dma_start(bq_sb[:], bq2[:])
            nc.sync.dma_start(bk_sb[:], bk2[:])
            nc.sync.dma_start(bv_sb[:], bvr[:])
            for j in range(4):
                t = wvp.tile([128, 4, 512], BF, tag="wvt", name="wvt")
                nc.sync.dma_start(t[:], wv4[j])
                wv_sb.extend(t[:, a, :] for a in range(4))
            for j in range(4):
                nc.sync.dma_start(et_tiles[j][:, :, 512:S], eT4[j][:, :, 512:S])
            for j in range(4):
                t = xtp.tile([128, 4, TOK], BF, tag="xt", name="xt")
                nc.scalar.dma_start(t[:], xT4[j])
                xT_sb.extend(t[:, a, :] for a in range(4))
            wq_blk = []
            for j in range(4):
                t = wqp.tile([128, 4, 512], BF, tag="wq", name="wqt")
                nc.scalar.dma_start(t[:], wq4[j][:, :, 0:512])
                wq_blk.append(t)
            nc.sync.dma_start(gam_sb[:], gamr[:])
            nc.sync.dma_start(bet_sb[:], betr[:])
            if use_mask:
                mask_sb = []
                for sc in range(SC):
                    t = mkp.tile([128, TOK], BF, tag="mk", name="mk")
                    nc.sync.dma_start(t[:], maskT[sc * 128:(sc + 1) * 128, :])
                    mask_sb.append(t)

            def k_proj(sh):
                # k-major across 4 concurrent PSUM chains: the PE consumes
                # each arriving (wk, eT) chunk with 4 matmuls, so the first
                # projection is DMA-paced with no per-chain stalls
                pss = [psKV.tile([128, 512], FP, tag="psKV", name="psKV")
                       for _ in range(4)]
                for k in range(KC):
                    for m in range(4):
                        nc.tensor.matmul(
                            pss[m][:],
                            wk_sb[k][:, m * 128:(m + 1) * 128],
                            et_tiles[k // 4][:, k % 4, sh * 512:(sh + 1) * 512],
                            start=(k == 0),
                            stop=(k == KC - 1),
                        )
                for m in range(4):
                    for hi in range(2):
                        h = 2 * m + hi
                        pb = hi * 64
                        nc.vector.tensor_scalar_add(
                            kT_sb[h][:, sh * 512:(sh + 1) * 512],
                            pss[m][pb:pb + 64, :],
                            bk_sb[pb:pb + 64, m:m + 1],
                        )

            def v_proj(scs):
                scs = list(scs)
                pss = [psKV.tile([128, 512], FP, tag="psKV", name="psKV")
                       for _ in scs]
                for k in range(KC):
                    for i, sc in enumerate(scs):
                        nc.tensor.matmul(
                            pss[i][:],
                            et_tiles[k // 4][:, k % 4, sc * 128:(sc + 1) * 128],
                            wv_sb[k],
                            start=(k == 0),
                            stop=(k == KC - 1),
                        )
                for i, sc in enumerate(scs):
                    v = vvp.tile([128, KVH, HD + 1], BF, tag="vv", name="vv")
                    nc.vector.tensor_add(
                        v[:, :, 0:HD],
                        pss[i].rearrange("p (h d) -> p h d", d=HD),
                        bv_sb.rearrange("p (h d) -> p h d", d=HD),
                    )
                    nc.vector.memset(v[:, :, HD:HD + 1], 1.0)
                    vv_sb.append(v)

            # DMA-arrival-paced: K(lo) -> V(lo) -> K(hi) -> V(hi)
            k_proj(0)
            v_proj(range(0, 4))
            k_proj(1)
            v_proj(range(4, SC))


        # inputs eT/wk/wv released; open attention pools in their space
        psS = _stk.enter_context(tc.tile_pool(name="psS", bufs=2, space="PSUM"))
        psO = _stk.enter_context(tc.tile_pool(name="psO", bufs=2, space="PSUM"))
        expp = _stk.enter_context(tc.tile_pool(name="expp", bufs=26))
        recp = _stk.enter_context(tc.tile_pool(name="rec", bufs=4))
        bcp = _stk.enter_context(tc.tile_pool(name="bc", bufs=4))
        ones_sb = constp.tile([1, 64], BF, name="ones_sb")
        nc.vector.memset(ones_sb[:], 1.0)

        def q_proj(m):
            q = m % 4
            ps = psA.tile([128, TOK], FP, tag="psA", name="psA")
            for k in range(KC):
                nc.tensor.matmul(
                    ps[:],
                    wq_blk[k // 4][:, k % 4, q * 128:(q + 1) * 128],
                    xT_sb[k][:],
                    start=(k == 0),
                    stop=(k == KC - 1),
                )
            for hi in range(2):
                qt = qtp.tile([64, TOK], BF, tag="qt", name="qt")
                nc.vector.tensor_scalar_add(
                    qt[:], ps[hi * 64:hi * 64 + 64, :],
                    bq_sb[hi * 64:hi * 64 + 64, m:m + 1],
                )
                qT_sb[2 * m + hi] = qt

        def scores_gp(h, gp):
            tiles = []
            for sc in range(SC):
                lhs = kT_sb[h][:, sc * 128:(sc + 1) * 128]
                ps = psS.tile([128, 2, TOK], FP, tag="psS", name="psS")
                for gi in range(2):
                    hh = h * G + gp * 2 + gi
                    nc.tensor.matmul(
                        ps[:, gi, :], lhs, qT_sb[hh][:], start=True, stop=True
                    )
                    if use_mask:
                        nc.vector.tensor_add(
                            ps[:, gi, :], ps[:, gi, :], mask_sb[sc][:]
                        )
                ex = expp.tile([128, 2, TOK], BF, tag="ex", name="ex")
                nc.scalar.activation(ex[:], ps[:], func=Exp, scale=0.125)
                tiles.append(ex)
            return tiles

        def attn_v_pair(h, gp, tiles):
            for gi in range(2):
                g = gp * 2 + gi
                hh = h * G + g
                po = psO.tile([128, TOK], FP, tag="psO", name="psO")
                for sc in range(SC):
                    nc.tensor.matmul(
                        po[0:HD + 1, :],
                        vv_sb[sc][:, h, :],
                        tiles[sc][:, gi, :],
                        start=(sc == 0),
                        stop=(sc == SC - 1),
                    )
                recb = recp.tile([1, TOK], BF, tag="recb", name="recb")
                with nc.allow_low_precision(reason="softmax recip rounds to bf16"):
                    nc.vector.reciprocal(recb[:], po[HD:HD + 1, :])
                po_sb = bcp.tile([64, TOK], FP, tag="posb", name="posb")
                nc.vector.tensor_copy(po_sb[:], po[0:HD, :])
                # broadcast recip across 64 partitions with a K=1 matmul into
                # the free upper half of the same PSUM bank
                nc.tensor.matmul(
                    po[64:128, :], ones_sb[:], recb[:], start=True, stop=True
                )
                nc.vector.tensor_mul(
                    ctx_sb[hh // 2][(hh % 2) * 64:(hh % 2) * 64 + 64, :],
                    po_sb[:],
                    po[64:128, :],
                )

        wo_blk0 = []
        pending = None
        for nb in range(4):
            h0, h1 = 2 * nb, 2 * nb + 1
            if pending is not None:
                attn_v_pair(*pending)  # (h1, gp1) of the previous nb
                pending = None
            q_proj(4 * nb + 0)
            q_proj(4 * nb + 1)
            t0 = scores_gp(h0, 0)
            q_proj(4 * nb + 2)
            q_proj(4 * nb + 3)
            if nb < 3:
                nxt = []
                for j in range(4):
                    t = wqp.tile([128, 4, 512], BF, tag="wq", name="wqt")
                    nc.scalar.dma_start(
                        t[:], wq4[j][:, :, (nb + 1) * 512:(nb + 2) * 512]
                    )
                    nxt.append(t)
            t1 = scores_gp(h0, 1)
            u0 = scores_gp(h1, 0)
            attn_v_pair(h0, 0, t0)
            if nb == 3:
                for nb2 in range(2):  # prefetch Wo blocks 0 and 1
                    for j in range(4):
                        t = wqp.tile([128, 4, 512], BF, tag="wq", name="wqt")
                        nc.scalar.dma_start(
                            t[:], wo4[j][:, :, nb2 * 512:(nb2 + 1) * 512]
                        )
                        wo_blk0.append(t)
            u1 = scores_gp(h1, 1)
            attn_v_pair(h0, 1, t1)
            attn_v_pair(h1, 0, u0)
            pending = (h1, 1, u1)
            if nb < 3:
                wq_blk = nxt
        attn_v_pair(*pending)

        _stk.close()  # release attention-phase pools

        # ---- Phase C: output projection + residual + layernorm ------------
        # tt-outer so each row-block's LN/store overlaps the next block's mms
        with (
            tc.tile_pool(name="psC", bufs=4, space="PSUM") as psC,
            tc.tile_pool(name="wC2", bufs=12) as wcp2,
            tc.tile_pool(name="xr", bufs=6) as xrp,
            tc.tile_pool(name="outp", bufs=2) as outp,
        ):
            def _ln(tt, ob):
                stats = lnp.tile([128, 4, 6], FP, tag="st", name="st")
                for sg in range(4):
                    nc.vector.bn_stats(
                        stats[:, sg, :], ob[:, sg * 512:(sg + 1) * 512]
                    )
                mv = lnp.tile([128, 2], FP, tag="mv", name="mv")
                nc.vector.bn_aggr(mv[:], stats[:])
                std = lnp.tile([128, 1], FP, tag="sd", name="sd")
                nc.scalar.activation(
                    std[:], mv[:, 1:2], func=Sqrt, bias=eps_sb[:], scale=1.0
                )
                rstd = lnp.tile([128, 1], FP, tag="rs", name="rs")
                nc.vector.reciprocal(rstd[:], std[:])
                nc.vector.tensor_scalar(
                    ob[:],
                    ob[:],
                    scalar1=mv[:, 0:1],
                    scalar2=rstd[:],
                    op0=mybir.AluOpType.subtract,
                    op1=mybir.AluOpType.mult,
                )
                eng = nc.vector if tt == 3 else nc.gpsimd
                eng.tensor_mul(ob[:], ob[:], gam_sb[:])
                eng.tensor_add(ob[:], ob[:], bet_sb[:])
                nc.sync.dma_start(out[tt * 128:(tt + 1) * 128, :], ob[:])

            wo_blks = [
                [wo_blk0[k // 4][:, k % 4, :] for k in range(KC)],
                [wo_blk0[4 + k // 4][:, k % 4, :] for k in range(KC)],
            ]
            for nb in range(2, 4):
                blk = []
                for j in range(4):
                    t = wcp2.tile([128, 4, 512], BF, tag="wo", name="wot")
                    nc.scalar.dma_start(
                        t[:], wo4[j][:, :, nb * 512:(nb + 1) * 512]
                    )
                    blk.extend(t[:, a, :] for a in range(4))
                wo_blks.append(blk)

            for tt in range(4):
                ob = outp.tile([128, H], FP, tag="ob", name="ob")
                for nb in range(4):
                    xt = xrp.tile([128, 512], FP, tag="xr", name="xr")
                    nc.sync.dma_start(
                        xt[:],
                        xres[tt * 128:(tt + 1) * 128, nb * 512:(nb + 1) * 512],
                    )
                    ps = psC.tile([128, 512], FP, tag="psC", name="psC")
                    for k in range(KC):
                        nc.tensor.matmul(
                            ps[:],
                            ctx_sb[k][:, tt * 128:(tt + 1) * 128],
                            wo_blks[nb][k][:],
                            start=(k == 0),
                            stop=(k == KC - 1),
                        )
                    sl = slice(nb * 512, (nb + 1) * 512)
                    nc.vector.tensor_add(ob[:, sl], ps[:], xt[:])
                _ln(tt, ob)


        wqp_cm.__exit__(None, None, None)

    nc.compile()
    return nc


def _get_nc(use_mask: bool):
    if use_mask not in _CACHE:
        _CACHE[use_mask] = _build(use_mask)
    return _CACHE[use_mask]


def kernel(
    hidden_state,
    encoder_hidden_state,
    encoder_attention_mask,
    Wq, bq, Wk, bk, Wv, bv, Wo, bo, gamma, beta,
):
    hidden_state = np.asarray(hidden_state, dtype=np.float32)
    encoder_hidden_state = np.asarray(encoder_hidden_state, dtype=np.float32)
    encoder_attention_mask = np.asarray(encoder_attention_mask, dtype=np.float32)
    Wq = np.asarray(Wq, dtype=np.float32)
    bq = np.asarray(bq, dtype=np.float32)
    Wk = np.asarray(Wk, dtype=np.float32)
    bk = np.asarray(bk, dtype=np.float32)
    Wv = np.asarray(Wv, dtype=np.float32)
    bv = np.asarray(bv, dtype=np.float32)
    Wo = np.asarray(Wo, dtype=np.float32)
    bo = np.asarray(bo, dtype=np.float32)
    gamma = np.asarray(gamma, dtype=np.float32)
    beta = np.asarray(beta, dtype=np.float32)

    use_mask = bool(np.any(encoder_attention_mask))
    nc = _get_nc(use_mask)
    in_maps = _prepare_in_maps(
        hidden_state, encoder_hidden_state, encoder_attention_mask,
        Wq, bq, Wk, bk, Wv, bv, Wo, bo, gamma, beta, use_mask,
    )

    res = run_bass_kernel_spmd(nc, in_maps, core_ids=list(range(8)))
    kernel._last_results = res

    output = np.empty((B, L, H), dtype=np.float32)
    for c in range(8):
        b, lh = c // 2, c % 2
        output[b, lh * TOK:(lh + 1) * TOK, :] = res.results[c]["out"]
    return output


def _prepare_in_maps(
    hidden_state, encoder_hidden_state, encoder_attention_mask,
    Wq, bq, Wk, bk, Wv, bv, Wo, bo, gamma, beta, use_mask,
):
    wq_bf = np.ascontiguousarray(Wq.astype(BF16))
    wk_bf = np.ascontiguousarray(Wk.astype(BF16))
    wv_bf = np.ascontiguousarray(Wv.astype(BF16))
    wo_bf = np.ascontiguousarray(Wo.astype(BF16))
    bq2 = np.ascontiguousarray(bq.reshape(KC, 128).T)
    bk2 = np.ascontiguousarray(bk.reshape(4, 128).T)
    bvr = np.ascontiguousarray(np.tile(bv[None, :], (128, 1)))
    gamr = np.ascontiguousarray(np.tile(gamma[None, :].astype(BF16), (128, 1)))
    betr = np.ascontiguousarray(np.tile(beta[None, :].astype(BF16), (128, 1)))

    eT_by_b = [
        np.ascontiguousarray(encoder_hidden_state[b].T.astype(BF16)) for b in range(B)
    ]

    in_maps = []
    for c in range(8):
        b, lh = c // 2, c % 2
        rows = hidden_state[b, lh * TOK:(lh + 1) * TOK, :]
        m = {
            "xT": np.ascontiguousarray(rows.T.astype(BF16)),
            "xres": rows + bo[None, :],
            "eT": eT_by_b[b],
            "wq": wq_bf, "wk": wk_bf, "wv": wv_bf, "wo": wo_bf,
            "bq2": bq2, "bk2": bk2, "bvr": bvr,
            "gamr": gamr, "betr": betr,
        }
        if use_mask:
            mslice = encoder_attention_mask[b, 0, lh * TOK:(lh + 1) * TOK, :]
            m["maskT"] = np.ascontiguousarray((mslice.T * 8.0).astype(BF16))
        in_maps.append(m)
    return in_maps



# revision 6
# speedup vs baseline: 1.4575x; 1.4575x over previous
"""Encoder-decoder GQA attention block (B=4, L=S=1024, H=2048, 32 Q heads,
8 KV heads, head_dim 64) + output projection + residual + layernorm, on 8
Trainium2 NeuronCores.

Sharding: rows. Core c handles batch c//2, L-half c%2 (512 query rows).
K/V projections per-batch (duplicated on the 2 cores sharing a batch),
attention over all 32 heads for the core's rows, output projection,
residual + LN. No collectives.

fp8 DoubleRow build: all large matmuls run as fp8e4 (e4m3) DoubleRow pairs
(0.5 cycles/output-row, 2x contraction per instruction => ~4x vs bf16 for
K>=256 chains, 2x for the K=64 score matmuls via a zero-padded second pair
half). Softmax exp runs on ACT (psum fp32 -> fp8, scale 1/8, bias -2 to
center the fp8 range; shift cancels in normalization) and is the bottleneck
engine (~133us busy). Per-head softmax normalization: rowsum via a
ones-column appended to V (M padded to 96: DoubleRow ldweights needs
32-aligned M), reciprocal on DVE, partition-broadcast of the reciprocal row
via a stride-0 SBUF->SBUF DMA on the GPSIMD (SWDGE) queue, then one DVE
multiply psum x sbuf -> fp8 ctx. Residual+LN in bf16.

Layouts: contraction operands are packed host-side as [128, chunk, 2, N]
(partition-major DoubleRow pairs, contraction index = (c*2+i)*128+p).
kT is stored [64, 9, 128] with chunk 8 zeroed so score matmuls pair chunk
sc with zeros via step-sliced APs kt[:, sc::(8-sc), :]; qT [64, 5, 512]
with a junk (finite) 5th slot paired via qt[:, g::(4-g), :].
"""

from contextlib import ExitStack

import numpy as np
import ml_dtypes

import concourse.bass as bass  # noqa: F401
import concourse.mybir as mybir
import concourse.tile as tile
from concourse import bacc
from concourse.bass_utils import run_bass_kernel_spmd

BF16 = ml_dtypes.bfloat16
NPF8 = ml_dtypes.float8_e4m3

H = 2048
NH = 32
KVH = 8
G = 4
HD = 64
B, L, S = 4, 1024, 1024
TOK = 512
EPS = 1e-6

FP = mybir.dt.float32
BF = mybir.dt.bfloat16
F8 = mybir.dt.float8e4
DR = mybir.MatmulPerfMode.DoubleRow

_CACHE: dict = {}


def _build(use_mask: bool):
    nc = bacc.Bacc("TRN2", target_bir_lowering=False)

    xt8 = nc.dram_tensor("xt8", [128, 8, 2, TOK], F8, kind="ExternalInput")
    et8 = nc.dram_tensor("et8", [128, 8, 2, S], F8, kind="ExternalInput")
    wk8 = nc.dram_tensor("wk8", [128, 8, 2, 512], F8, kind="ExternalInput")
    wv8 = nc.dram_tensor("wv8", [128, 8, 2, 512], F8, kind="ExternalInput")
    wq8 = nc.dram_tensor("wq8", [128, 4, 8, 2, 512], F8, kind="ExternalInput")
    wo8 = nc.dram_tensor("wo8", [128, 4, 8, 2, 512], F8, kind="ExternalInput")
    xres = nc.dram_tensor("xres", [4, 128, H], BF, kind="ExternalInput")
    bq2 = nc.dram_tensor("bq2", [128, 16], FP, kind="ExternalInput")
    bk2 = nc.dram_tensor("bk2", [128, 4], FP, kind="ExternalInput")
    bvr = nc.dram_tensor("bvr", [128, KVH, HD], BF, kind="ExternalInput")
    gamr = nc.dram_tensor("gamr", [128, H], BF, kind="ExternalInput")
    betr = nc.dram_tensor("betr", [128, H], BF, kind="ExternalInput")
    identd = nc.dram_tensor("identd", [128, 128], BF, kind="ExternalInput")
    if use_mask:
        maskT = nc.dram_tensor("maskT", [8, 128, TOK], BF, kind="ExternalInput")
    out = nc.dram_tensor("out", [4, 128, H], BF, kind="ExternalOutput")

    Exp = mybir.ActivationFunctionType.Exp
    Sqrt = mybir.ActivationFunctionType.Sqrt

    with tile.TileContext(nc) as tc:
      with (
          tc.tile_pool(name="cc", bufs=1) as ccp,
          tc.tile_pool(name="kt", bufs=KVH) as ktp,
          tc.tile_pool(name="qt", bufs=KVH) as qtp,
          tc.tile_pool(name="vv", bufs=4) as vvp,
          tc.tile_pool(name="ctx", bufs=8) as ctxp,
          tc.tile_pool(name="wo", bufs=4) as wop,
          tc.tile_pool(name="ln", bufs=10) as lnp,
          tc.tile_pool(name="psA", bufs=2, space="PSUM") as psA,
      ):
        gam_sb = ccp.tile([128, H], BF, name="gam_sb")
        bet_sb = ccp.tile([128, H], BF, name="bet_sb")
        eps_sb = ccp.tile([128, 1], FP, name="eps_sb")
        nbias = ccp.tile([128, 1], FP, name="nbias")
        bq_sb = ccp.tile([128, 16], FP, name="bq_sb")
        bk_sb = ccp.tile([128, 4], FP, name="bk_sb")
        bv_sb = ccp.tile([128, KVH, HD], BF, name="bv_sb")
        nc.vector.memset(eps_sb[:], EPS)
        nc.vector.memset(nbias[:], -2.0)

        kt_sb = [ktp.tile([64, 9, 128], F8, name="kt") for _ in range(KVH)]
        qt_sb = [qtp.tile([64, 5, TOK], F8, name="qt") for _ in range(KVH)]
        vv_sb = [vvp.tile([128, 2, KVH, 96], F8, name="vv") for _ in range(4)]
        ctx_sb = [ctxp.tile([128, 2, TOK], F8, name="ctx") for _ in range(8)]
        for h in range(KVH):
            nc.gpsimd.memset(kt_sb[h][:, 8, :], 0.0)
            nc.gpsimd.memset(qt_sb[h][:, 4, :], 0.0)
        for j in range(4):
            nc.gpsimd.memset(vv_sb[j][:, :, :, 64:65], 1.0)
            nc.gpsimd.memset(vv_sb[j][:, :, :, 65:96], 0.0)

        _stk = ExitStack()
        xtp = _stk.enter_context(tc.tile_pool(name="xtp", bufs=1))
        wqp = _stk.enter_context(tc.tile_pool(name="wqp", bufs=2))
        mkp = _stk.enter_context(
            tc.tile_pool(name="maskp", bufs=8 if use_mask else 1))

        xt_sb = xtp.tile([128, 8, 2, TOK], F8, name="xt_sb")
        wq_cur = wqp.tile([128, 8, 2, 512], F8, tag="wq", name="wq")

        with (
            tc.tile_pool(name="eTp", bufs=1) as etp,
            tc.tile_pool(name="wkv", bufs=1) as wkvp,
            tc.tile_pool(name="psKV", bufs=4, space="PSUM") as psKV,
        ):
            et_sb = etp.tile([128, 8, 2, S], F8, name="et_sb")
            wk_sb = wkvp.tile([128, 8, 2, 512], F8, name="wk_sb")
            wv_sb = wkvp.tile([128, 8, 2, 512], F8, name="wv_sb")

            # input DMAs, needed-first (all on SP)
            nc.sync.dma_start(wk_sb[:], wk8[:])
            nc.sync.dma_start(et_sb[:, :, :, 0:512], et8[:, :, :, 0:512])
            nc.sync.dma_start(xt_sb[:], xt8[:])
            nc.sync.dma_start(wq_cur[:], wq8[:, 0])
            nc.sync.dma_start(et_sb[:, :, :, 512:S], et8[:, :, :, 512:S])
            nc.sync.dma_start(wv_sb[:], wv8[:])
            nc.sync.dma_start(bq_sb[:], bq2[:])
            nc.sync.dma_start(bk_sb[:], bk2[:])
            nc.sync.dma_start(bv_sb[:], bvr[:])
            if use_mask:
                mask_sb = []
                for sc in range(8):
                    t = mkp.tile([128, TOK], BF, tag="mk", name="mk")
                    nc.sync.dma_start(t[:], maskT[sc])
                    mask_sb.append(t)
            nc.sync.dma_start(gam_sb[:], gamr[:])
            nc.sync.dma_start(bet_sb[:], betr[:])

            def k_proj(sh):
                pss = [psKV.tile([128, 512], FP, tag="psKV", name="psKV")
                       for _ in range(4)]
                for c in range(8):
                    for m in range(4):
                        nc.tensor.matmul(
                            pss[m][:],
                            wk_sb[:, c, :, m * 128:(m + 1) * 128],
                            et_sb[:, c, :, sh * 512:(sh + 1) * 512],
                            start=(c == 0), stop=(c == 7), perf_mode=DR,
                        )
                for m in range(4):
                    for hi in range(2):
                        h = 2 * m + hi
                        pb = hi * 64
                        nc.vector.tensor_scalar_add(
                            kt_sb[h][:, 4 * sh:4 * sh + 4, :],
                            pss[m][pb:pb + 64, :],
                            bk_sb[pb:pb + 64, m:m + 1],
                        )

            def v_proj(scs):
                scs = list(scs)
                pss = [psKV.tile([128, 512], FP, tag="psKV", name="psKV")
                       for _ in scs]
                for c in range(8):
                    for i, sc in enumerate(scs):
                        nc.tensor.matmul(
                            pss[i][:],
                            et_sb[:, c, :, sc * 128:(sc + 1) * 128],
                            wv_sb[:, c, :, :],
                            start=(c == 0), stop=(c == 7), perf_mode=DR,
                        )
                for i, sc in enumerate(scs):
                    nc.vector.tensor_add(
                        vv_sb[sc // 2][:, sc % 2, :, 0:HD],
                        pss[i].rearrange("p (h d) -> p h d", d=HD),
                        bv_sb[:],
                    )

            k_proj(0)
            v_proj(range(0, 4))
            k_proj(1)
            v_proj(range(4, 8))

        # attention-phase pools in the freed space
        psS = _stk.enter_context(tc.tile_pool(name="psS", bufs=2, space="PSUM"))
        psO = _stk.enter_context(tc.tile_pool(name="psO", bufs=2, space="PSUM"))
        expp = _stk.enter_context(tc.tile_pool(name="expp", bufs=12))
        recp = _stk.enter_context(tc.tile_pool(name="rec", bufs=4))
        rbp = _stk.enter_context(tc.tile_pool(name="rb", bufs=4))

        def q_proj(m):
            ps = psA.tile([128, TOK], FP, tag="psA", name="psA")
            for c in range(8):
                nc.tensor.matmul(
                    ps[:],
                    wq_cur[:, c, :, (m % 4) * 128:(m % 4 + 1) * 128],
                    xt_sb[:, c, :, :],
                    start=(c == 0), stop=(c == 7), perf_mode=DR,
                )
            h = m // 2
            with nc.allow_low_precision(reason="fp8 attention"):
                for hi in range(2):
                    g = 2 * (m % 2) + hi
                    nc.vector.tensor_scalar_add(
                        qt_sb[h][:, g, :],
                        ps[hi * 64:hi * 64 + 64, :],
                        bq_sb[hi * 64:hi * 64 + 64, m:m + 1],
                    )

        def scores_gp(h, gp):
            tiles = []
            ex = None
            for sc in range(8):
                ps = psS.tile([128, 2, TOK], FP, tag="psS", name="psS")
                for gi in range(2):
                    g = gp * 2 + gi
                    nc.tensor.matmul(
                        ps[:, gi, :],
                        kt_sb[h][:, sc::(8 - sc), :],
                        qt_sb[h][:, g::(4 - g), :],
                        start=True, stop=True, perf_mode=DR,
                    )
                    if use_mask:
                        nc.vector.tensor_add(
                            ps[:, gi, :], ps[:, gi, :], mask_sb[sc][:]
                        )
                if sc % 2 == 0:
                    ex = expp.tile([128, 2, 2, TOK], F8, tag="ex", name="ex")
                    tiles.append(ex)
                with nc.allow_low_precision(reason="fp8 attention"):
                    nc.scalar.activation(
                        ex[:, sc % 2, :, :], ps[:],
                        func=Exp, scale=0.125, bias=nbias[:],
                    )
            return tiles

        def attn_v(h, gp, tiles):
            for gi in range(2):
                g = gp * 2 + gi
                hh = h * G + g
                po = psO.tile([128, TOK], FP, tag="psO", name="psO")
                for j in range(4):
                    nc.tensor.matmul(
                        po[0:96, :],
                        vv_sb[j][:, :, h, :],
                        tiles[j][:, :, gi, :],
                        start=(j == 0), stop=(j == 3), perf_mode=DR,
                    )
                rec = recp.tile([1, TOK], BF, tag="rec", name="rec")
                r64 = rbp.tile([64, TOK], BF, tag="r64", name="r64")
                with nc.allow_low_precision(reason="fp8 attention"):
                    nc.vector.reciprocal(rec[:], po[64:65, :])
                    nc.gpsimd.partition_broadcast(r64[:], rec[:], channels=64)
                    c, i2, pb = hh // 4, (hh % 4) // 2, ((hh % 4) % 2) * 64
                    nc.vector.tensor_mul(
                        ctx_sb[c][pb:pb + 64, i2, :], po[0:64, :], r64[:]
                    )

        wo_sb = []
        pending = None
        for nb in range(4):
            h0, h1 = 2 * nb, 2 * nb + 1
            if pending is not None:
                attn_v(*pending)
                pending = None
            q_proj(4 * nb + 0)
            q_proj(4 * nb + 1)
            t0 = scores_gp(h0, 0)
            q_proj(4 * nb + 2)
            q_proj(4 * nb + 3)
            if nb < 3:
                nxt = wqp.tile([128, 8, 2, 512], F8, tag="wq", name="wq")
                for cq in range(4):
                    nc.sync.dma_start(
                        nxt[:, 2 * cq:2 * cq + 2, :, :],
                        wq8[:, nb + 1, 2 * cq:2 * cq + 2, :, :],
                    )
            t1 = scores_gp(h0, 1)
            u0 = scores_gp(h1, 0)
            attn_v(h0, 0, t0)
            if nb == 2:
                for onb in range(4):
                    t = wop.tile([128, 8, 2, 512], F8, name="wo_sb")
                    for cq in range(4):
                        nc.sync.dma_start(
                            t[:, 2 * cq:2 * cq + 2, :, :],
                            wo8[:, onb, 2 * cq:2 * cq + 2, :, :],
                        )
                    wo_sb.append(t)
            u1 = scores_gp(h1, 1)
            attn_v(h0, 1, t1)
            attn_v(h1, 0, u0)
            pending = (h1, 1, u1)
            if nb < 3:
                wq_cur = nxt
        attn_v(*pending)

        _stk.close()

        # ---- Phase C: output projection + residual + layernorm ------------
        with (
            tc.tile_pool(name="psC", bufs=4, space="PSUM") as psC,
            tc.tile_pool(name="xr", bufs=2) as xrp,
            tc.tile_pool(name="outp", bufs=2) as outp,
        ):
            def _ln(tt, ob):
                stats = lnp.tile([128, 4, 6], FP, tag="st", name="st")
                for sg in range(4):
                    nc.vector.bn_stats(
                        stats[:, sg, :], ob[:, sg * 512:(sg + 1) * 512]
                    )
                mv = lnp.tile([128, 2], FP, tag="mv", name="mv")
                nc.vector.bn_aggr(mv[:], stats[:])
                std = lnp.tile([128, 1], FP, tag="sd", name="sd")
                nc.scalar.activation(
                    std[:], mv[:, 1:2], func=Sqrt, bias=eps_sb[:], scale=1.0
                )
                rstd = lnp.tile([128, 1], FP, tag="rs", name="rs")
                nc.vector.reciprocal(rstd[:], std[:])
                with nc.allow_low_precision(reason="bf16 layernorm"):
                    nc.vector.tensor_scalar(
                        ob[:], ob[:],
                        scalar1=mv[:, 0:1], scalar2=rstd[:],
                        op0=mybir.AluOpType.subtract,
                        op1=mybir.AluOpType.mult,
                    )
                    nc.vector.tensor_mul(ob[:], ob[:], gam_sb[:])
                    nc.vector.tensor_add(ob[:], ob[:], bet_sb[:])
                nc.sync.dma_start(out[tt], ob[:])

            for tt in range(4):
                xr_sb = xrp.tile([128, H], BF, tag="xr", name="xr")
                nc.sync.dma_start(xr_sb[:], xres[tt])
                ob = outp.tile([128, H], BF, tag="ob", name="ob")
                for nb in range(4):
                    ps = psC.tile([128, 512], FP, tag="psC", name="psC")
                    for c in range(8):
                        nc.tensor.matmul(
                            ps[:],
                            ctx_sb[c][:, :, tt * 128:(tt + 1) * 128],
                            wo_sb[nb][:, c, :, :],
                            start=(c == 0), stop=(c == 7), perf_mode=DR,
                        )
                    sl = slice(nb * 512, (nb + 1) * 512)
                    with nc.allow_low_precision(reason="bf16 residual"):
                        nc.vector.tensor_add(ob[:, sl], ps[:], xr_sb[:, sl])
                _ln(tt, ob)

    nc.compile()
    return nc


def _get_nc(use_mask: bool):
    if use_mask not in _CACHE:
        _CACHE[use_mask] = _build(use_mask)
    return _CACHE[use_mask]


def _pack_pairs(w):
    """[2048, N] -> [128, 8, 2, N] with contraction index (c*2+i)*128+p."""
    n = w.shape[1]
    return np.ascontiguousarray(
        w.reshape(8, 2, 128, n).transpose(2, 0, 1, 3).astype(NPF8)
    )


def _prepare_in_maps(
    hidden_state, encoder_hidden_state, encoder_attention_mask,
    Wq, bq, Wk, bk, Wv, bv, Wo, bo, gamma, beta, use_mask,
):
    wk8 = _pack_pairs(Wk)
    wv8 = _pack_pairs(Wv)
    wq8 = np.ascontiguousarray(
        Wq.reshape(8, 2, 128, 4, 512).transpose(2, 3, 0, 1, 4).astype(NPF8)
    )
    wo8 = np.ascontiguousarray(
        Wo.reshape(8, 2, 128, 4, 512).transpose(2, 3, 0, 1, 4).astype(NPF8)
    )
    bq2 = np.ascontiguousarray(bq.reshape(16, 128).T)
    bk2 = np.ascontiguousarray(bk.reshape(4, 128).T)
    bvr = np.ascontiguousarray(
        np.tile(bv.reshape(1, KVH, HD), (128, 1, 1)).astype(BF16))
    gamr = np.ascontiguousarray(np.tile(gamma[None, :].astype(BF16), (128, 1)))
    betr = np.ascontiguousarray(np.tile(beta[None, :].astype(BF16), (128, 1)))

    et8_by_b = [_pack_pairs(encoder_hidden_state[b].T) for b in range(B)]

    in_maps = []
    for c in range(8):
        b, lh = c // 2, c % 2
        rows = hidden_state[b, lh * TOK:(lh + 1) * TOK, :]
        m = {
            "xt8": _pack_pairs(rows.T),
            "et8": et8_by_b[b],
            "wk8": wk8, "wv8": wv8, "wq8": wq8, "wo8": wo8,
            "xres": np.ascontiguousarray(
                (rows + bo[None, :]).reshape(4, 128, H).astype(BF16)),
            "bq2": bq2, "bk2": bk2, "bvr": bvr,
            "gamr": gamr, "betr": betr,
        }
        if use_mask:
            mslice = encoder_attention_mask[b, 0, lh * TOK:(lh + 1) * TOK, :]
            m["maskT"] = np.ascontiguousarray(
                (mslice.T * 8.0).reshape(8, 128, TOK).astype(BF16))
        in_maps.append(m)
    return in_maps


def kernel(
    hidden_state,
    encoder_hidden_state,
    encoder_attention_mask,
    Wq, bq, Wk, bk, Wv, bv, Wo, bo, gamma, beta,
):
    hidden_state = np.asarray(hidden_state, dtype=np.float32)
    encoder_hidden_state = np.asarray(encoder_hidden_state, dtype=np.float32)
    encoder_attention_mask = np.asarray(encoder_attention_mask, dtype=np.float32)
    Wq = np.asarray(Wq, dtype=np.float32)
    bq = np.asarray(bq, dtype=np.float32)
    Wk = np.asarray(Wk, dtype=np.float32)
    bk = np.asarray(bk, dtype=np.float32)
    Wv = np.asarray(Wv, dtype=np.float32)
    bv = np.asarray(bv, dtype=np.float32)
    Wo = np.asarray(Wo, dtype=np.float32)
    bo = np.asarray(bo, dtype=np.float32)
    gamma = np.asarray(gamma, dtype=np.float32)
    beta = np.asarray(beta, dtype=np.float32)

    use_mask = bool(np.any(encoder_attention_mask))
    nc = _get_nc(use_mask)
    in_maps = _prepare_in_maps(
        hidden_state, encoder_hidden_state, encoder_attention_mask,
        Wq, bq, Wk, bk, Wv, bv, Wo, bo, gamma, beta, use_mask,
    )

    res = run_bass_kernel_spmd(nc, in_maps, core_ids=list(range(8)))
    kernel._last_results = res

    output = np.empty((B, L, H), dtype=np.float32)
    for c in range(8):
        b, lh = c // 2, c % 2
        output[b, lh * TOK:(lh + 1) * TOK, :] = (
            res.results[c]["out"].astype(np.float32).reshape(TOK, H)
        )
    return output
